# revision 36
# baseline (speedup 1.0000x reference)
"""Trainium2 Bass kernel for segmented min/max + MLP (MinMaxDiffSetFeat).

Computation (reference):
    seg = row -> segment id from CSR pointers
    h = concat([x, x - seg_min[seg], x - seg_max[seg]], 1) @ w1 -> lrelu -> @ w2 -> lrelu

Device strategy (per core, data-parallel over segments):
  - Host splits segments across 8 cores at CSR boundaries near N/8 multiples.
  - Host sends x transposed and stacked: partitions 0-63 = x^T, 64-127 = -x^T.
  - Segmented min via tensor_tensor_scan with reset: state = min(state + r, x)
    where r = +BIG at segment starts. Fwd scan + bwd scan (negative-stride APs),
    combined with elementwise min => per-element full-segment min. The -x half
    yields -seg_max for free in the same [128, F] scan.
  - MLP decomposition: h@w1 = x@(A+B+C) - seg_min@B - seg_max@C  (w1=[A;B;C]),
    so three K=64 accumulating matmuls + lrelu + one more matmul + lrelu.
  - Chunks are self-contained via halo H >= max segment length.
"""

import os
import sys

import numpy as np

for _p in ("/opt/trn_rl_repo",):
    if _p not in sys.path and os.path.isdir(_p):
        sys.path.insert(0, _p)

N = 500_000
D = 64
M = 8          # cores
F = 4096       # chunk center width (rows per chunk)
SL = 512       # matmul slice width (one PSUM bank)
RPEN = 16.0    # per-boundary penalty for the min-plus segmented scan;
               # must exceed the value range of x (randn: |x| < ~6)

LAST_EXEC_NS = None
LAST_RESULTS = None

_module_cache = {}
_last_geom = None  # (RP, H) of the most recent _prepare


_dve_ops = None


def _register_custom_dve_ops():
    """Segmented prefix/suffix min at 1 elem/cycle via the custom-DVE scan
    facility (the stock tensor_tensor_scan pays a multi-cycle feedback
    bubble per element).  Min-plus trick: with c = cumsum of per-boundary
    penalties (same row for every partition, exact f32 multiples of RPEN),

        fwd:  pref_k = min_{j<=k}(x_j - c_j) + c_k
        bwd:  comb_k = min_{j>=k}(pref_j + c_j) - c_k

    equal the segmented prefix/suffix minima because crossing a boundary
    costs +RPEN > range(x).  The bwd pass reuses the SAME c and just runs
    with reversed access patterns."""
    global _dve_ops
    if _dve_ops is not None:
        return _dve_ops
    import numpy as np_

    from concourse import dve_ops as DOPS
    from concourse.dve_spec import C0, Spec, Src0, Src1, lower, scan
    from concourse.dve_uop import AluOp, DveOpSpec

    def _mk(name, body, ref):
        for o in DOPS.OPS:
            if o.name == name:
                return o
        DOPS._SUB_OPCODE_FOR_NAME[name] = (
            max(DOPS._SUB_OPCODE_FOR_NAME.values()) + 1)
        spec = Spec(body=body, reference=ref)
        shas = {}
        for ver in ("v3", "v4"):
            try:
                uops = lower(spec, ver=ver)
            except Exception:
                continue
            s = DveOpSpec(name=name,
                          opcode=DOPS._SUB_OPCODE_FOR_NAME[name],
                          uops=uops, rd1_en=True)
            shas[ver] = s.sha(ver)
        op = DOPS.DveOp(name, spec, subdim=False, uops_sha=shas)
        DOPS.OPS.append(op)
        DOPS.CUSTOM_DVE_SPECS[name] = spec
        return op

    fwd = _mk(
        "ANT_SEGMIN_FWD",
        scan(AluOp.MIN, Src0 - Src1, init=C0) + Src1,
        lambda in0, in1, s0, s1, imm2: (
            np_.minimum(np_.minimum.accumulate(
                in0.astype(np_.float32) - in1, axis=-1), s0) + in1
        ).astype(np_.float32),
    )
    bwd = _mk(
        "ANT_SEGMIN_BWD",
        scan(AluOp.MIN, Src0 + Src1, init=C0) - Src1,
        lambda in0, in1, s0, s1, imm2: (
            np_.minimum(np_.minimum.accumulate(
                in0.astype(np_.float32) + in1, axis=-1), s0) - in1
        ).astype(np_.float32),
    )
    _dve_ops = (fwd, bwd)
    return _dve_ops


def _build_module(RP, H, bench_iters=1):
    import concourse.mybir as mybir
    from concourse import bacc
    from concourse.tile import TileContext

    FH = F + 2 * H
    n_chunks = RP // F
    lim = int(os.environ.get("KERNEL_NCHUNKS", "0"))
    if lim:
        n_chunks = min(n_chunks, lim)
    S = F // SL
    seg_fwd, seg_bwd = _register_custom_dve_ops()

    nc = bacc.Bacc("TRN2")
    xs = nc.dram_tensor("xs", [128, RP + 2 * H], mybir.dt.bfloat16,
                        kind="ExternalInput")
    rst = nc.dram_tensor("rst", [RP // F, FH], mybir.dt.float32,
                         kind="ExternalInput")
    wp = nc.dram_tensor("wp", [128, 256], mybir.dt.bfloat16,
                        kind="ExternalInput")
    yT = nc.dram_tensor("yT", [64, RP], mybir.dt.float32,
                        kind="ExternalOutput")

    f32 = mybir.dt.float32
    bf16 = mybir.dt.bfloat16
    lrelu = mybir.ActivationFunctionType.Prelu
    FLT_MAX = 3.4e38

    # DMA queue split to avoid head-of-line blocking: input loads (rst row,
    # xs chunk) ride the sync HWDGE ring so they can run ahead of compute;
    # output stores ride the scalar/ACT ring, issued in program order right
    # after ACT writes yo (no extra semaphore); only the rs partition
    # broadcast stays on the gpsimd SWDGE ring. DVE probes absorb the
    # xx/rst/PE semaphores before the scans need them; ACT's leaky2 is
    # traced after the NEXT slice's leaky1 so its PE dependency is always
    # pre-observed.
    with TileContext(nc) as tc:
        with tc.tile_pool(name="wpool", bufs=1) as wpool, \
             tc.tile_pool(name="data", bufs=3) as dpool, \
             tc.tile_pool(name="mmio", bufs=6) as mpool, \
             tc.tile_pool(name="probe", bufs=1) as prpool, \
             tc.tile_pool(name="psum", bufs=3, space="PSUM") as ppool:
            wtd = wpool.tile([128, 256], bf16, tag="wtd")
            nc.gpsimd.dma_start(out=wtd[:], in_=wp[:, :])
            wt = wpool.tile([128, 256], bf16, tag="wt")
            nc.vector.tensor_copy(wt[:], wtd[:])
            bias = wpool.tile([64, 1], f32, tag="bias")
            nc.vector.memset(bias[:], 0.0)
            alpha = wpool.tile([64, 1], f32, tag="alpha")
            nc.vector.memset(alpha[:], 0.2)
            warm = wpool.tile([64, 1], f32, tag="warm")
            nc.scalar.copy(warm[:], bias[:])

            # bench_iters > 1: unroll the whole chunk sweep bench_iters times
            # inside one NEFF so a single execution runs the kernel that many
            # times back to back; the timing slope between two bench_iters
            # values is then pure steady-state device time per execution.
            prev = None      # (h1 tile, output offset) awaiting tail
            last_ps = None   # last PSUM tile of previous chunk (PE absorber)
            for k in [kk for _ in range(bench_iters)
                      for kk in range(n_chunks)]:
                c0 = k * F
                cc = dpool.tile([128, FH], f32, tag="cc")
                nc.sync.dma_start(out=cc[0:1, :], in_=rst[k:k + 1, :])
                nc.gpsimd.dma_start(
                    out=cc[1:128, :],
                    in_=cc[0:1, :].unsqueeze(1).broadcast_to([1, 127, FH]))
                xx = dpool.tile([128, FH], bf16, tag="xx")
                nc.sync.dma_start(out=xx[:], in_=xs[:, c0:c0 + FH])

                pb1 = prpool.tile([1, 1], f32, tag=f"pb1_{k}")
                nc.vector.tensor_copy(pb1[:], xx[0:1, 0:1])
                pb2 = prpool.tile([1, 1], f32, tag=f"pb2_{k}")
                nc.vector.tensor_copy(pb2[:], cc[0:1, 0:1])
                if last_ps is not None:
                    pb3 = prpool.tile([1, 1], f32, tag=f"pb3_{k}")
                    nc.vector.tensor_copy(pb3[:], last_ps[0:1, 0:1])

                pref = dpool.tile([128, FH], bf16, tag="pref")
                nc.vector._custom_dve(seg_fwd, out=pref[:], in0=xx[:],
                                      in1=cc[:], s0=FLT_MAX)
                # suffix-min over pref == full segment min at every element
                comb = dpool.tile([128, FH], bf16, tag="comb")
                nc.vector._custom_dve(seg_bwd, out=comb[:, ::-1],
                                      in0=pref[:, ::-1], in1=cc[:, ::-1],
                                      s0=FLT_MAX)

                if os.environ.get("KERNEL_STAGE", "") == "scan":
                    # scan-isolation bench: no matmul/act/output stages
                    continue
                for s in range(S):
                    sl = slice(H + s * SL, H + (s + 1) * SL)
                    cs = slice(s * SL, (s + 1) * SL)
                    # tail of previous slice first: keeps PE order
                    # mm2_{s-1} < mm1_s and ACT order l1_s after, so l2's
                    # PE need is covered by l1's wait.
                    ps1 = ppool.tile([64, SL], f32, tag="ps1")
                    if prev is not None:
                        h1p, offp = prev
                        ps2 = ppool.tile([64, SL], f32, tag="ps2")
                        nc.tensor.matmul(ps2[:], wt[0:64, 192:256], h1p[:],
                                         start=True, stop=True)
                    nc.tensor.matmul(ps1[:], wt[0:64, 0:64],
                                     xx[0:64, sl],
                                     start=True, stop=False)
                    nc.tensor.matmul(ps1[:], wt[0:128, 64:128],
                                     comb[0:128, sl],
                                     start=False, stop=True)
                    h1 = mpool.tile([64, SL], bf16, tag="h1")
                    if os.environ.get("KERNEL_STAGE", "") == "noact":
                        nc.vector.tensor_copy(h1[:], ps1[:])
                    else:
                        nc.scalar.activation(h1[:], ps1[:], lrelu,
                                             bias[:, 0:1], alpha=alpha[:, 0:1])
                    if prev is not None:
                        yo = mpool.tile([64, SL], f32, tag="yo")
                        if os.environ.get("KERNEL_STAGE", "") == "noact":
                            nc.vector.tensor_copy(yo[:], ps2[:])
                        else:
                            nc.scalar.activation(yo[:], ps2[:], lrelu,
                                                 bias[:, 0:1], alpha=alpha[:, 0:1])
                        nc.scalar.dma_start(
                            out=yT[:, offp:offp + SL], in_=yo[:])
                        last_ps = ps2
                    prev = (h1, c0 + s * SL)
            # drain the last slice
            if prev is not None:
                h1p, offp = prev
                ps2 = ppool.tile([64, SL], f32, tag="ps2")
                nc.tensor.matmul(ps2[:], wt[0:64, 192:256], h1p[:],
                                 start=True, stop=True)
                yo = mpool.tile([64, SL], f32, tag="yo")
                nc.scalar.activation(yo[:], ps2[:], lrelu, bias[:, 0:1],
                                     alpha=alpha[:, 0:1])
                nc.scalar.dma_start(out=yT[:, offp:offp + SL], in_=yo[:])
    nc.finalize()
    return nc


def _prepare(inputs):
    x = np.ascontiguousarray(np.asarray(inputs["x"], dtype=np.float32))
    csr = np.asarray(inputs["csr_idx"]).astype(np.int64)
    w1 = np.asarray(inputs["w1"], dtype=np.float32)
    w2 = np.asarray(inputs["w2"], dtype=np.float32)
    n, d = x.shape
    assert d == D

    # --- segment-aligned core cuts near k*n/M ---
    cuts = [0]
    for kk in range(1, M):
        target = kk * n // M
        gi = int(np.searchsorted(csr, target))
        lo = csr[gi - 1] if gi > 0 else 0
        hi = csr[gi] if gi < len(csr) else n
        cuts.append(int(hi if hi - target <= target - lo else lo))
    cuts.append(n)

    Rs = [cuts[i + 1] - cuts[i] for i in range(M)]
    Rmax = max(Rs)
    n_chunks = (Rmax + F - 1) // F
    RP = n_chunks * F

    seglen = np.diff(csr)
    Lmax = int(seglen.max()) if len(seglen) else 1
    H = 128
    while H < Lmax:
        H *= 2

    # RPEN at the first row of every non-empty segment (duplicates collapse)
    is_start = np.zeros(n, dtype=np.float32)
    starts = csr[:-1]
    starts = starts[starts < n]
    is_start[starts] = RPEN

    import ml_dtypes
    bf16 = ml_dtypes.bfloat16

    wpack = np.zeros((128, 256), dtype=np.float32)
    wpack[0:64, 0:64] = w1[0:64] + w1[64:128] + w1[128:192]   # Wsum
    wpack[0:64, 64:128] = -w1[64:128]                          # -B (seg_min)
    wpack[64:128, 64:128] = w1[128:192]                        # C (-seg_max)
    wpack[0:64, 192:256] = w2
    wpack = wpack.astype(bf16)

    FH = F + 2 * H
    n_chunks = RP // F
    in_maps = []
    for c in range(M):
        r0, r1 = cuts[c], cuts[c + 1]
        R = r1 - r0
        xsb = np.zeros((128, RP + 2 * H), dtype=np.float32)
        xT = x[r0:r1].T
        xsb[0:64, H:H + R] = xT
        xsb[64:128, H:H + R] = -xT
        # boundary-penalty vector; every pad position is its own segment
        rsv = np.full((RP + 2 * H,), RPEN, dtype=np.float64)
        rsv[H:H + R] = is_start[r0:r1]
        # per-chunk inclusive cumsum rows (window-normalized so the f32
        # magnitudes stay small; values are exact multiples of RPEN)
        crow = np.empty((n_chunks, FH), dtype=np.float32)
        for kk in range(n_chunks):
            crow[kk] = np.cumsum(rsv[kk * F:kk * F + FH]).astype(np.float32)
        in_maps.append({"xs": xsb.astype(bf16), "rst": crow, "wp": wpack})

    global _last_geom
    _last_geom = (RP, H)
    nc = _get_module(RP, H, 1)
    return nc, in_maps, cuts, n


def _get_module(RP, H, bench_iters):
    key = (RP, H, bench_iters)
    if key not in _module_cache:
        _module_cache[key] = _build_module(RP, H, bench_iters)
    return _module_cache[key]


def kernel(**inputs):
    global LAST_EXEC_NS, LAST_RESULTS
    from concourse.bass_utils import run_bass_kernel_spmd

    nc, in_maps, cuts, n = _prepare(inputs)
    trace = os.environ.get("KERNEL_TRACE", "0") == "1"
    ncores = int(os.environ.get("KERNEL_CORES", str(M)))
    res = run_bass_kernel_spmd(nc, in_maps[:ncores],
                               core_ids=list(range(ncores)), trace=trace)
    LAST_EXEC_NS = res.exec_time_ns
    LAST_RESULTS = res

    out = np.empty((n, D), dtype=np.float32)
    for c in range(len(res.results)):
        r0, r1 = cuts[c], cuts[c + 1]
        out[r0:r1] = res.results[c]["yT"][:, :r1 - r0].T
    return out


def _make_runner(nc, in_maps):
    """jit a single-NEFF-execution dispatch for module nc; returns
    (fn, dev_in, make_dev_zero)."""
    import jax
    from jax.sharding import Mesh, NamedSharding, PartitionSpec
    from jax.experimental.shard_map import shard_map

    import concourse.mybir as mybir
    from concourse import bass2jax

    bass2jax.install_neuronx_cc_hook()

    partition_name = (nc.partition_id_tensor.name
                      if nc.partition_id_tensor else None)
    in_names, out_names, out_avals = [], [], []
    for alloc in nc.m.functions[0].allocations:
        if not isinstance(alloc, mybir.MemoryLocationSet):
            continue
        name = alloc.memorylocations[0].name
        if alloc.kind == "ExternalInput":
            if name != partition_name:
                in_names.append(name)
        elif alloc.kind == "ExternalOutput":
            out_names.append(name)
            out_avals.append(jax.core.ShapedArray(
                tuple(alloc.tensor_shape), mybir.dt.np(alloc.dtype)))
    n_params = len(in_names)
    zero_shapes = [(M * a.shape[0], *a.shape[1:]) for a in out_avals]
    all_names = in_names + out_names
    if partition_name is not None:
        all_names.append(partition_name)

    def _body(*args):
        operands = list(args)
        if partition_name is not None:
            operands.append(bass2jax.partition_id_tensor())
        return tuple(bass2jax._bass_exec_p.bind(
            *operands,
            out_avals=tuple(out_avals),
            in_names=tuple(all_names),
            out_names=tuple(out_names),
            lowering_input_output_aliases=(),
            sim_require_finite=True,
            sim_require_nnan=True,
            nc=nc,
        ))

    devices = jax.devices()[:M]
    mesh = Mesh(np.asarray(devices), ("core",))
    n_outs = len(out_names)
    fn = jax.jit(shard_map(
        _body, mesh=mesh,
        in_specs=(PartitionSpec("core"),) * (n_params + n_outs),
        out_specs=(PartitionSpec("core"),) * n_outs,
        check_rep=False), keep_unused=True)

    sh = NamedSharding(mesh, PartitionSpec("core"))
    dev_in = [jax.device_put(
        np.concatenate([in_maps[c][nm] for c in range(M)], axis=0), sh)
        for nm in in_names]

    def make_dev_zero():
        return [jax.device_put(np.zeros(zs, np.float32), sh)
                for zs in zero_shapes]

    return fn, dev_in, make_dev_zero


def benchmark(n_reps=4, burst=32, k_lo=1, k_hi=7, **inputs):
    """Measure per-execution hardware time of the 8-core kernel.

    The axon-tunneled PJRT dispatch has ~60-90 ms of fixed host/network
    round-trip overhead plus ~1.5 ms of per-dispatch channel cost, orders
    of magnitude above the device time, so wall-clocking one dispatch
    measures the network, not the hardware.  Two corrections:

      1. Modules are built with the kernel body unrolled k times, so one
         NEFF execution runs the kernel k times back to back on device.
      2. Timing uses pipelined bursts (enqueue `burst` dispatches, block
         once), and the slope between the k_hi and k_lo modules
         (T_hi - T_lo) / (burst * (k_hi - k_lo)) cancels both the
         round-trip overhead and the per-dispatch channel cost.

    What remains is the marginal steady-state device time of one full
    kernel execution (input HBM DMA, compute, output HBM DMA).
    Returns seconds per kernel execution."""
    import time

    import jax

    _, in_maps, cuts, n = _prepare(inputs)
    RP, H = _last_geom
    runners = {}
    for k in (k_lo, k_hi):
        nck = _get_module(RP, H, bench_iters=k)
        fn, dev_in, make_dev_zero = _make_runner(nck, in_maps)
        # warmup (compile + first exec)
        out = fn(*dev_in, *make_dev_zero())
        jax.block_until_ready(out)
        runners[k] = (fn, dev_in, make_dev_zero)

    best = {k_lo: float("inf"), k_hi: float("inf")}
    # interleave k_lo/k_hi bursts so network drift cancels in the slope
    for _ in range(n_reps):
        for k in (k_lo, k_hi):
            fn, dev_in, make_dev_zero = runners[k]
            out = tuple(make_dev_zero())
            t0 = time.perf_counter()
            for _b in range(burst):
                out = fn(*dev_in, *out)
            jax.block_until_ready(out)
            best[k] = min(best[k], time.perf_counter() - t0)
    return (best[k_hi] - best[k_lo]) / (burst * (k_hi - k_lo))



# revision 38
# speedup vs baseline: 2.7414x; 2.7414x over previous
"""Trainium2 Bass kernel for segmented min/max + MLP (MinMaxDiffSetFeat).

Computation (reference):
    seg = row -> segment id from CSR pointers
    h = concat([x, x - seg_min[seg], x - seg_max[seg]], 1) @ w1 -> lrelu -> @ w2 -> lrelu

Device strategy (per core, data-parallel over segments):
  - Host splits segments across 8 cores at CSR boundaries near N/8 multiples.
  - Host sends x transposed and stacked in bf16: partitions 0-63 = x^T,
    64-127 = -x^T (the MLP tolerance allows bf16 data + weights end to end;
    PSUM accumulation and the final output stay fp32).
  - Segmented min via tensor_tensor_scan with reset: state = min(state + r, x)
    where r = +BIG at segment starts. Fwd scan + bwd scan (negative-stride APs)
    => per-element full-segment min. The -x half yields -seg_max for free in
    the same [128, FH] scan. The two DVE scans (~5.5 cycles/element in bf16)
    are the critical path; every other engine hides behind them.
  - MLP decomposition: h@w1 = x@(A+B+C) - seg_min@B - seg_max@C  (w1=[A;B;C]),
    so a K=64 + K=128 accumulating matmul pair + lrelu + one K=64 matmul +
    lrelu, sliced SL=512 wide (one PSUM bank).
  - DMA queues are split to avoid head-of-line blocking (inputs on the sync
    HWDGE ring, outputs on the scalar/ACT ring, rs broadcast on gpsimd).
  - Chunks are self-contained via halo H >= max segment length.
  - benchmark() measures true per-execution device time via the slope between
    modules whose body is unrolled k=1 vs k=7 times (see its docstring).
"""

import os
import sys

import numpy as np

for _p in ("/opt/trn_rl_repo",):
    if _p not in sys.path and os.path.isdir(_p):
        sys.path.insert(0, _p)

N = 500_000
D = 64
M = 8          # cores
F = 4096       # chunk center width (rows per chunk)
SL = 512       # matmul slice width (one PSUM bank)
BIG = 1e30

LAST_EXEC_NS = None
LAST_RESULTS = None

_module_cache = {}
_last_geom = None  # (RP, H) of the most recent _prepare


def _build_module(RP, H, bench_iters=1):
    import concourse.mybir as mybir
    from concourse import bacc
    from concourse.tile import TileContext

    FH = F + 2 * H
    n_chunks = RP // F
    lim = int(os.environ.get("KERNEL_NCHUNKS", "0"))
    if lim:
        n_chunks = min(n_chunks, lim)
    S = F // SL
    nc = bacc.Bacc("TRN2")
    xs = nc.dram_tensor("xs", [128, RP + 2 * H], mybir.dt.bfloat16,
                        kind="ExternalInput")
    rst = nc.dram_tensor("rst", [1, RP + 2 * H + 1], mybir.dt.bfloat16,
                         kind="ExternalInput")
    wp = nc.dram_tensor("wp", [128, 256], mybir.dt.bfloat16,
                        kind="ExternalInput")
    yT = nc.dram_tensor("yT", [64, RP], mybir.dt.float32,
                        kind="ExternalOutput")

    fmin = mybir.AluOpType.min
    fadd = mybir.AluOpType.add
    f32 = mybir.dt.float32
    bf16 = mybir.dt.bfloat16
    lrelu = mybir.ActivationFunctionType.Prelu

    # DMA queue split to avoid head-of-line blocking: input loads (rst row,
    # xs chunk) ride the sync HWDGE ring so they can run ahead of compute;
    # output stores ride the scalar/ACT ring, issued in program order right
    # after ACT writes yo (no extra semaphore); only the rs partition
    # broadcast stays on the gpsimd SWDGE ring. DVE probes absorb the
    # xx/rst/PE semaphores before the scans need them; ACT's leaky2 is
    # traced after the NEXT slice's leaky1 so its PE dependency is always
    # pre-observed.
    with TileContext(nc) as tc:
        with tc.tile_pool(name="wpool", bufs=1) as wpool, \
             tc.tile_pool(name="data", bufs=3) as dpool, \
             tc.tile_pool(name="mmio", bufs=6) as mpool, \
             tc.tile_pool(name="probe", bufs=1) as prpool, \
             tc.tile_pool(name="psum", bufs=3, space="PSUM") as ppool:
            wtd = wpool.tile([128, 256], bf16, tag="wtd")
            nc.gpsimd.dma_start(out=wtd[:], in_=wp[:, :])
            wt = wpool.tile([128, 256], bf16, tag="wt")
            nc.vector.tensor_copy(wt[:], wtd[:])
            bias = wpool.tile([64, 1], f32, tag="bias")
            nc.vector.memset(bias[:], 0.0)
            alpha = wpool.tile([64, 1], f32, tag="alpha")
            nc.vector.memset(alpha[:], 0.2)
            warm = wpool.tile([64, 1], f32, tag="warm")
            nc.scalar.copy(warm[:], bias[:])

            # bench_iters > 1: unroll the whole chunk sweep bench_iters times
            # inside one NEFF so a single execution runs the kernel that many
            # times back to back; the timing slope between two bench_iters
            # values is then pure steady-state device time per execution.
            prev = None      # (h1 tile, output offset) awaiting tail
            last_ps = None   # last PSUM tile of previous chunk (PE absorber)
            for k in [kk for _ in range(bench_iters)
                      for kk in range(n_chunks)]:
                c0 = k * F
                rs = dpool.tile([128, FH + 1], bf16, tag="rs")
                nc.sync.dma_start(out=rs[0:1, :], in_=rst[:, c0:c0 + FH + 1])
                nc.gpsimd.dma_start(
                    out=rs[1:128, :],
                    in_=rs[0:1, :].unsqueeze(1).broadcast_to([1, 127, FH + 1]))
                xx = dpool.tile([128, FH], bf16, tag="xx")
                nc.sync.dma_start(out=xx[:], in_=xs[:, c0:c0 + FH])

                pb1 = prpool.tile([1, 1], f32, tag=f"pb1_{k}")
                nc.vector.tensor_copy(pb1[:], xx[0:1, 0:1])
                pb2 = prpool.tile([1, 1], f32, tag=f"pb2_{k}")
                nc.vector.tensor_copy(pb2[:], rs[0:1, 0:1])
                if last_ps is not None:
                    pb3 = prpool.tile([1, 1], f32, tag=f"pb3_{k}")
                    nc.vector.tensor_copy(pb3[:], last_ps[0:1, 0:1])

                pref = dpool.tile([128, FH], bf16, tag="pref")
                nc.vector.tensor_tensor_scan(
                    out=pref[:], data0=rs[:, 0:FH], data1=xx[:],
                    initial=BIG, op0=fadd, op1=fmin)
                # suffix-min over pref == full segment min at every element
                comb = dpool.tile([128, FH], bf16, tag="comb")
                nc.vector.tensor_tensor_scan(
                    out=comb[:, ::-1], data0=rs[:, 1:FH + 1][:, ::-1],
                    data1=pref[:, ::-1], initial=BIG, op0=fadd, op1=fmin)

                if os.environ.get("KERNEL_STAGE", "") == "scan":
                    # scan-isolation bench: no matmul/act/output stages
                    continue
                for s in range(S):
                    sl = slice(H + s * SL, H + (s + 1) * SL)
                    cs = slice(s * SL, (s + 1) * SL)
                    # tail of previous slice first: keeps PE order
                    # mm2_{s-1} < mm1_s and ACT order l1_s after, so l2's
                    # PE need is covered by l1's wait.
                    ps1 = ppool.tile([64, SL], f32, tag="ps1")
                    if prev is not None:
                        h1p, offp = prev
                        ps2 = ppool.tile([64, SL], f32, tag="ps2")
                        nc.tensor.matmul(ps2[:], wt[0:64, 192:256], h1p[:],
                                         start=True, stop=True)
                    nc.tensor.matmul(ps1[:], wt[0:64, 0:64],
                                     xx[0:64, sl],
                                     start=True, stop=False)
                    nc.tensor.matmul(ps1[:], wt[0:128, 64:128],
                                     comb[0:128, sl],
                                     start=False, stop=True)
                    h1 = mpool.tile([64, SL], bf16, tag="h1")
                    if os.environ.get("KERNEL_STAGE", "") == "noact":
                        nc.vector.tensor_copy(h1[:], ps1[:])
                    else:
                        nc.scalar.activation(h1[:], ps1[:], lrelu,
                                             bias[:, 0:1], alpha=alpha[:, 0:1])
                    if prev is not None:
                        yo = mpool.tile([64, SL], f32, tag="yo")
                        if os.environ.get("KERNEL_STAGE", "") == "noact":
                            nc.vector.tensor_copy(yo[:], ps2[:])
                        else:
                            nc.scalar.activation(yo[:], ps2[:], lrelu,
                                                 bias[:, 0:1], alpha=alpha[:, 0:1])
                        nc.scalar.dma_start(
                            out=yT[:, offp:offp + SL], in_=yo[:])
                        last_ps = ps2
                    prev = (h1, c0 + s * SL)
            # drain the last slice
            if prev is not None:
                h1p, offp = prev
                ps2 = ppool.tile([64, SL], f32, tag="ps2")
                nc.tensor.matmul(ps2[:], wt[0:64, 192:256], h1p[:],
                                 start=True, stop=True)
                yo = mpool.tile([64, SL], f32, tag="yo")
                nc.scalar.activation(yo[:], ps2[:], lrelu, bias[:, 0:1],
                                     alpha=alpha[:, 0:1])
                nc.scalar.dma_start(out=yT[:, offp:offp + SL], in_=yo[:])
    nc.finalize()
    return nc


def _prepare(inputs):
    x = np.ascontiguousarray(np.asarray(inputs["x"], dtype=np.float32))
    csr = np.asarray(inputs["csr_idx"]).astype(np.int64)
    w1 = np.asarray(inputs["w1"], dtype=np.float32)
    w2 = np.asarray(inputs["w2"], dtype=np.float32)
    n, d = x.shape
    assert d == D

    # --- segment-aligned core cuts near k*n/M ---
    cuts = [0]
    for kk in range(1, M):
        target = kk * n // M
        gi = int(np.searchsorted(csr, target))
        lo = csr[gi - 1] if gi > 0 else 0
        hi = csr[gi] if gi < len(csr) else n
        cuts.append(int(hi if hi - target <= target - lo else lo))
    cuts.append(n)

    Rs = [cuts[i + 1] - cuts[i] for i in range(M)]
    Rmax = max(Rs)
    n_chunks = (Rmax + F - 1) // F
    RP = n_chunks * F

    seglen = np.diff(csr)
    Lmax = int(seglen.max()) if len(seglen) else 1
    H = 128
    while H < Lmax:
        H *= 2

    # BIG at the first row of every non-empty segment (duplicates collapse)
    is_start = np.zeros(n, dtype=np.float32)
    starts = csr[:-1]
    starts = starts[starts < n]
    is_start[starts] = BIG

    import ml_dtypes
    bf16 = ml_dtypes.bfloat16

    wpack = np.zeros((128, 256), dtype=np.float32)
    wpack[0:64, 0:64] = w1[0:64] + w1[64:128] + w1[128:192]   # Wsum
    wpack[0:64, 64:128] = -w1[64:128]                          # -B (seg_min)
    wpack[64:128, 64:128] = w1[128:192]                        # C (-seg_max)
    wpack[0:64, 192:256] = w2
    wpack = wpack.astype(bf16)

    in_maps = []
    for c in range(M):
        r0, r1 = cuts[c], cuts[c + 1]
        R = r1 - r0
        xsb = np.zeros((128, RP + 2 * H), dtype=np.float32)
        xT = x[r0:r1].T
        xsb[0:64, H:H + R] = xT
        xsb[64:128, H:H + R] = -xT
        rstb = np.full((RP + 2 * H + 1,), BIG, dtype=np.float32)
        rstb[H:H + R] = is_start[r0:r1]
        in_maps.append({"xs": xsb.astype(bf16),
                        "rst": rstb[None, :].astype(bf16), "wp": wpack})

    global _last_geom
    _last_geom = (RP, H)
    nc = _get_module(RP, H, 1)
    return nc, in_maps, cuts, n


def _get_module(RP, H, bench_iters):
    key = (RP, H, bench_iters)
    if key not in _module_cache:
        _module_cache[key] = _build_module(RP, H, bench_iters)
    return _module_cache[key]


def kernel(**inputs):
    global LAST_EXEC_NS, LAST_RESULTS
    from concourse.bass_utils import run_bass_kernel_spmd

    nc, in_maps, cuts, n = _prepare(inputs)
    trace = os.environ.get("KERNEL_TRACE", "0") == "1"
    ncores = int(os.environ.get("KERNEL_CORES", str(M)))
    res = run_bass_kernel_spmd(nc, in_maps[:ncores],
                               core_ids=list(range(ncores)), trace=trace)
    LAST_EXEC_NS = res.exec_time_ns
    LAST_RESULTS = res

    out = np.empty((n, D), dtype=np.float32)
    for c in range(len(res.results)):
        r0, r1 = cuts[c], cuts[c + 1]
        out[r0:r1] = res.results[c]["yT"][:, :r1 - r0].T
    return out


def _make_runner(nc, in_maps):
    """jit a single-NEFF-execution dispatch for module nc; returns
    (fn, dev_in, make_dev_zero)."""
    import jax
    from jax.sharding import Mesh, NamedSharding, PartitionSpec
    from jax.experimental.shard_map import shard_map

    import concourse.mybir as mybir
    from concourse import bass2jax

    bass2jax.install_neuronx_cc_hook()

    partition_name = (nc.partition_id_tensor.name
                      if nc.partition_id_tensor else None)
    in_names, out_names, out_avals = [], [], []
    for alloc in nc.m.functions[0].allocations:
        if not isinstance(alloc, mybir.MemoryLocationSet):
            continue
        name = alloc.memorylocations[0].name
        if alloc.kind == "ExternalInput":
            if name != partition_name:
                in_names.append(name)
        elif alloc.kind == "ExternalOutput":
            out_names.append(name)
            out_avals.append(jax.core.ShapedArray(
                tuple(alloc.tensor_shape), mybir.dt.np(alloc.dtype)))
    n_params = len(in_names)
    zero_shapes = [(M * a.shape[0], *a.shape[1:]) for a in out_avals]
    all_names = in_names + out_names
    if partition_name is not None:
        all_names.append(partition_name)

    def _body(*args):
        operands = list(args)
        if partition_name is not None:
            operands.append(bass2jax.partition_id_tensor())
        return tuple(bass2jax._bass_exec_p.bind(
            *operands,
            out_avals=tuple(out_avals),
            in_names=tuple(all_names),
            out_names=tuple(out_names),
            lowering_input_output_aliases=(),
            sim_require_finite=True,
            sim_require_nnan=True,
            nc=nc,
        ))

    devices = jax.devices()[:M]
    mesh = Mesh(np.asarray(devices), ("core",))
    n_outs = len(out_names)
    fn = jax.jit(shard_map(
        _body, mesh=mesh,
        in_specs=(PartitionSpec("core"),) * (n_params + n_outs),
        out_specs=(PartitionSpec("core"),) * n_outs,
        check_rep=False), keep_unused=True)

    sh = NamedSharding(mesh, PartitionSpec("core"))
    dev_in = [jax.device_put(
        np.concatenate([in_maps[c][nm] for c in range(M)], axis=0), sh)
        for nm in in_names]

    def make_dev_zero():
        return [jax.device_put(np.zeros(zs, np.float32), sh)
                for zs in zero_shapes]

    return fn, dev_in, make_dev_zero


def benchmark(n_reps=4, burst=32, k_lo=1, k_hi=7, **inputs):
    """Measure per-execution hardware time of the 8-core kernel.

    The axon-tunneled PJRT dispatch has ~60-90 ms of fixed host/network
    round-trip overhead plus ~1.5 ms of per-dispatch channel cost, orders
    of magnitude above the device time, so wall-clocking one dispatch
    measures the network, not the hardware.  Two corrections:

      1. Modules are built with the kernel body unrolled k times, so one
         NEFF execution runs the kernel k times back to back on device.
      2. Timing uses pipelined bursts (enqueue `burst` dispatches, block
         once), and the slope between the k_hi and k_lo modules
         (T_hi - T_lo) / (burst * (k_hi - k_lo)) cancels both the
         round-trip overhead and the per-dispatch channel cost.

    What remains is the marginal steady-state device time of one full
    kernel execution (input HBM DMA, compute, output HBM DMA).
    Returns seconds per kernel execution."""
    import time

    import jax

    _, in_maps, cuts, n = _prepare(inputs)
    RP, H = _last_geom
    runners = {}
    for k in (k_lo, k_hi):
        nck = _get_module(RP, H, bench_iters=k)
        fn, dev_in, make_dev_zero = _make_runner(nck, in_maps)
        # warmup (compile + first exec)
        out = fn(*dev_in, *make_dev_zero())
        jax.block_until_ready(out)
        runners[k] = (fn, dev_in, make_dev_zero)

    best = {k_lo: float("inf"), k_hi: float("inf")}
    # interleave k_lo/k_hi bursts so network drift cancels in the slope
    for _ in range(n_reps):
        for k in (k_lo, k_hi):
            fn, dev_in, make_dev_zero = runners[k]
            out = tuple(make_dev_zero())
            t0 = time.perf_counter()
            for _b in range(burst):
                out = fn(*dev_in, *out)
            jax.block_until_ready(out)
            best[k] = min(best[k], time.perf_counter() - t0)
    return (best[k_hi] - best[k_lo]) / (burst * (k_hi - k_lo))

